# revision 19
# baseline (speedup 1.0000x reference)
"""Fused DDiT transformer block (causal) on 8 TRN2 NeuronCores.

Sharding: attention is head-parallel (2 heads/core, 16 total) with QKV
column-sliced per core; two half AllToAlls (one per local head) re-shard
from head-split to token-split, and out-proj + MLP run token-parallel
(512 tokens/core).  The first AllToAll overlaps the second head's
attention compute.

LayerNorm gains are folded into the following matmul weights on the host.
LN1: stats come from a vector-engine tree reduction (x, x^2 over the 8
k-blocks) followed by tiny ones-column matmuls; centering is folded into
the QKV matmul via an appended K=1 rank-1 update and 1/std folds into the
RoPE tables (q,k) / a PSUM-eviction multiply (v).  LN2 is applied by
centering+scaling the activations once (broadcast rows via PE).  rstd is
computed with a single scalar-engine Rsqrt.  Softmax uses wide
[128,1024] exponentials spanning two PSUM banks and a fast-approx DVE
reciprocal; score and PV matmuls are software-pipelined one group ahead
so the PE never sits behind the scalar engine's exp.
Compute dtype bf16 (fp32 accumulation); the residual stream stays fp32.
"""
import sys

for _p in ("/opt/trn_rl_repo",):
    if _p not in sys.path:
        sys.path.append(_p)

import numpy as np
import ml_dtypes

import concourse.bass as bass
import concourse.tile as tile
import concourse.mybir as mybir
from concourse.bass_utils import run_bass_kernel_spmd
from concourse.masks import make_identity

bf16 = mybir.dt.bfloat16
f32 = mybir.dt.float32
AF = mybir.ActivationFunctionType
OP = mybir.AluOpType

N_CORES = 8
B, S, D = 2, 2048, 1024
T = B * S            # 4096 tokens total
NH, HD = 16, 64      # heads, head dim
HPC = NH // N_CORES  # 2 heads per core
TOK = T // N_CORES   # 512 tokens per core in the token-split phase
NCH = T // 1024      # 4 chunks of 1024 tokens in the QKV phase
LN_EPS = 1e-5

# ---------------------------------------------------------------------------
# Sync legalizer: this walrus build accepts only ONE sync wait and ONE sync
# update per TPB instruction. Move extras onto same-engine NoOps (engines
# complete instructions in program order, so semantics are preserved).
# ---------------------------------------------------------------------------
_uid = [0]


def _legalize_sync(nc):
    for f in nc.m.functions:
        for bb in f.blocks:
            out = []
            changed = False
            for inst in bb.instructions:
                # this walrus build cannot encode EVENT_SEMAPHORE_RANGE_CLEAR
                # ("ISA wrong length"); the kernel ends at a full barrier and
                # each NEFF execution gets fresh semaphores, so drop it but
                # keep its waits on a NoOp
                if isinstance(inst, mybir.InstISA) and (
                        getattr(inst, "ant_dict", None) or {}).get("range_first") is not None:
                    _uid[0] += 1
                    nop = mybir.InstNoOp(name=f"rcstrip-{_uid[0]}", ins=[], outs=[])
                    nop.engine = inst.engine
                    nop.sync_info = inst.sync_info
                    inst = nop
                    changed = True
                si = inst.sync_info
                if si is None:
                    out.append(inst)
                    continue
                waits = list(si.on_wait) if si.on_wait else []
                updates = list(si.on_update) if si.on_update else []
                if len(waits) <= 1 and len(updates) <= 1:
                    out.append(inst)
                    continue
                changed = True
                for w in waits[:-1]:
                    _uid[0] += 1
                    nop = mybir.InstNoOp(name=f"syncw-{_uid[0]}", ins=[], outs=[])
                    nop.engine = inst.engine
                    nop.sync_info = mybir.SyncInfo(on_wait=[w], on_update=[])
                    out.append(nop)
                inst.sync_info = mybir.SyncInfo(
                    on_wait=waits[-1:], on_update=updates[:1]
                )
                out.append(inst)
                for u in updates[1:]:
                    _uid[0] += 1
                    nop = mybir.InstNoOp(name=f"syncu-{_uid[0]}", ins=[], outs=[])
                    nop.engine = inst.engine
                    nop.sync_info = mybir.SyncInfo(on_wait=[], on_update=[u])
                    out.append(nop)
            if changed:
                bb.instructions = out
    return nc


# ---------------------------------------------------------------------------
# Kernel graph
# ---------------------------------------------------------------------------
def _build():
    nc = bass.Bass()

    # -- external inputs (per core)
    xT_blk = nc.dram_tensor("xT_blk", (NCH, 128, 8, 1024), bf16, kind="ExternalInput")
    xT_own = nc.dram_tensor("xT_own", (D, TOK), f32, kind="ExternalInput")
    wqkv_blk = nc.dram_tensor("wqkv_blk", (3, 128, 8, 128), bf16, kind="ExternalInput")
    wqkv_rs = nc.dram_tensor("wqkv_rs", (3, 1, 128), bf16, kind="ExternalInput")
    tab = nc.dram_tensor("tab", (2, 128, T), bf16, kind="ExternalInput")  # cos, sin(signed)
    wout_blk = nc.dram_tensor("wout_blk", (8, 128, 8, 128), bf16, kind="ExternalInput")
    w1_blk = nc.dram_tensor("w1_blk", (32, 128, 8, 128), bf16, kind="ExternalInput")
    b1_all_d = nc.dram_tensor("b1_all", (128, 32), f32, kind="ExternalInput")
    w2_blk = nc.dram_tensor("w2_blk", (8, 128, 32, 128), bf16, kind="ExternalInput")
    b2_all_d = nc.dram_tensor("b2_all", (128, 8), f32, kind="ExternalInput")
    out_d = nc.dram_tensor("out", (D, TOK), f32, kind="ExternalOutput")

    # -- internal DRAM (two half AllToAlls, one per local head)
    cc_in = [nc.dram_tensor(f"cc_in{h}", (N_CORES, 64, TOK), bf16, kind="Internal")
             for h in range(2)]
    cc_out = [nc.dram_tensor(f"cc_out{h}", (N_CORES, 64, TOK), bf16, kind="Internal")
              for h in range(2)]

    with tile.TileContext(nc) as tc, \
         nc.allow_low_precision(reason="bf16 block compute"):
        with tc.tile_pool(name="const", bufs=1) as pconst, \
             tc.tile_pool(name="work", bufs=3) as pwork:
            ident_bf = pconst.tile([128, 128], bf16)
            make_identity(nc, ident_bf)
            mask128 = pconst.tile([128, 128], bf16)
            nc.gpsimd.memset(mask128, 1.0)
            # causal: keep (1.0) where q_local - k_local = f - p >= 0
            nc.gpsimd.affine_select(
                out=mask128, in_=mask128, pattern=[[1, 128]],
                compare_op=OP.is_ge, fill=0.0, base=0, channel_multiplier=-1)
            ones_row = pconst.tile([1, 128], bf16)
            nc.vector.memset(ones_row, 1.0)
            ones_col = pconst.tile([128, 1], bf16)
            nc.vector.memset(ones_col, 1.0)
            eps_col = pconst.tile([128, 1], f32)
            nc.vector.memset(eps_col, LN_EPS)

            # =============================================================
            # Phase A/B fused: per-chunk LN1 stats (vector tree + tiny PE
            # matmuls) + QKV + RoPE + V
            # =============================================================
            attn_pool_cm = tc.tile_pool(name="attn", bufs=1)
            pattn = attn_pool_cm.__enter__()
            negmu_row = pattn.tile([1, T], bf16)
            rstd_row = pattn.tile([1, T], bf16)

            # QKV-phase streaming pool, freed before the MLP phase needs SBUF
            big_pool_cm = tc.tile_pool(name="big2", bufs=2)
            pbig2 = big_pool_cm.__enter__()

            # first x chunk before the big rope tables (shorter t=0 stall)
            xrts = {}

            def _load_chunk(ch):
                xr_t = pbig2.tile([128, 8, 1024], bf16, name=f"xr_{ch}",
                                  tag="xTr", bufs=2)
                nc.sync.dma_start(out=xr_t, in_=xT_blk[ch])
                xrts[ch] = xr_t

            _load_chunk(0)
            # persistent QKV weight tiles + rope tables
            wq_sb = {}
            rs_sb = {}
            for m in range(3):
                w = pconst.tile([128, 8, 128], bf16, name=f"wqkv_{m}",
                                tag=f"wqkv_{m}")
                nc.sync.dma_start(out=w, in_=wqkv_blk[m])
                wq_sb[m] = w
                r = pconst.tile([1, 128], bf16, name=f"wqkvrs_{m}",
                                tag=f"wqkvrs_{m}")
                nc.sync.dma_start(out=r, in_=wqkv_rs[m])
                rs_sb[m] = r
            tabs = []
            for ti in range(2):
                raw = pattn.tile([128, T], bf16, name=f"tab{ti}",
                                 tag=f"tab{ti}")
                nc.sync.dma_start(out=raw, in_=tab[ti])
                tabs.append(raw)
            tab_c, tab_s = tabs
            _load_chunk(1)

            qT_sb = pattn.tile([128, T], bf16)
            kT_sb = pattn.tile([128, T], bf16)
            v_all = pattn.tile([128, T // 128, 130], bf16)
            nc.vector.memset(v_all[:, :, 64:65], 1.0)
            nc.vector.memset(v_all[:, :, 129:130], 1.0)

            with tc.tile_pool(name="psQKV", bufs=2, space="PSUM") as psQ, \
                 tc.tile_pool(name="psVT", bufs=1, space="PSUM") as psVT, \
                 tc.tile_pool(name="psST", bufs=1, space="PSUM") as psST, \
                 tc.tile_pool(name="psBC", bufs=1, space="PSUM") as psBC:
                for ch in range(NCH):
                    chsl = slice(ch * 1024, (ch + 1) * 1024)
                    xrt = xrts[ch]
                    # LN1 stats: square + tree-reduce over the 8 k-blocks on
                    # the vector engine, then one ones-column matmul per stat
                    xsq = pbig2.tile([128, 8, 1024], bf16, tag="xsq", bufs=1)
                    nc.vector.tensor_mul(out=xsq, in0=xrt, in1=xrt)
                    r4x = pwork.tile([128, 4, 1024], bf16, tag="r4x", bufs=1)
                    nc.vector.tensor_add(out=r4x, in0=xrt[:, 0:4, :], in1=xrt[:, 4:8, :])
                    r2x = pwork.tile([128, 2, 1024], bf16, tag="r2x", bufs=1)
                    nc.vector.tensor_add(out=r2x, in0=r4x[:, 0:2, :], in1=r4x[:, 2:4, :])
                    xr1 = pwork.tile([128, 1024], bf16, tag="xr1", bufs=2)
                    nc.vector.tensor_add(out=xr1, in0=r2x[:, 0, :], in1=r2x[:, 1, :])
                    r4s = pwork.tile([128, 4, 1024], bf16, name="r4s", tag="r4x", bufs=1)
                    nc.vector.tensor_add(out=r4s, in0=xsq[:, 0:4, :], in1=xsq[:, 4:8, :])
                    r2s = pwork.tile([128, 2, 1024], bf16, name="r2s", tag="r2x", bufs=1)
                    nc.vector.tensor_add(out=r2s, in0=r4s[:, 0:2, :], in1=r4s[:, 2:4, :])
                    sr1 = pwork.tile([128, 1024], bf16, tag="sr1", bufs=2)
                    nc.vector.tensor_add(out=sr1, in0=r2s[:, 0, :], in1=r2s[:, 1, :])

                    rsb = pwork.tile([128, 1024], bf16, tag="rsb", bufs=2)
                    for half in range(2):
                        hs = slice(ch * 1024 + half * 512, ch * 1024 + half * 512 + 512)
                        lo, hi = half * 512, half * 512 + 512
                        ps_x = psST.tile([1, 512], f32, tag="stx", bufs=1)
                        nc.tensor.matmul(ps_x, ones_col, xr1[:, lo:hi],
                                         start=True, stop=True)
                        ps_q = psST.tile([1, 512], f32, tag="stq", bufs=1)
                        nc.tensor.matmul(ps_q, ones_col, sr1[:, lo:hi],
                                         start=True, stop=True)
                        nc.vector.tensor_scalar_mul(out=negmu_row[0:1, hs],
                                                    in0=ps_x, scalar1=-1.0 / D)
                        mus_r = pwork.tile([1, 512], f32, tag="mus1_r", bufs=2)
                        nc.scalar.activation(out=mus_r, in_=negmu_row[0:1, hs],
                                             func=AF.Square)
                        var_r = pwork.tile([1, 512], f32, tag="var1_r", bufs=2)
                        nc.vector.scalar_tensor_tensor(
                            out=var_r, in0=ps_q, scalar=1.0 / D, in1=mus_r,
                            op0=OP.mult, op1=OP.subtract)
                        # rstd = exp(-0.5*ln(var+eps)) — scalar engine only,
                        # same act-table set as the attention exp
                        lnv = pwork.tile([1, 512], f32, tag="lnv", bufs=2)
                        nc.scalar.activation(out=lnv, in_=var_r,
                                             func=AF.Ln, bias=eps_col[0:1, :])
                        nc.scalar.activation(out=rstd_row[0:1, hs], in_=lnv,
                                             func=AF.Exp, scale=-0.5)
                        ps_b = psBC.tile([128, 512], f32, tag="bc", bufs=1)
                        nc.tensor.matmul(ps_b, ones_row[0:1, 0:128],
                                         rstd_row[0:1, hs], start=True, stop=True)
                        nc.scalar.activation(out=rsb[:, lo:hi], in_=ps_b,
                                             func=AF.Copy)
                        nc.vector.tensor_mul(out=tab_c[:, hs], in0=tab_c[:, hs],
                                             in1=rsb[:, lo:hi])
                        nc.vector.tensor_mul(out=tab_s[:, hs], in0=tab_s[:, hs],
                                             in1=rsb[:, lo:hi])

                    for m in range(3):
                        ps = psQ.tile([128, 1024], f32, tag="qkv")
                        for half in range(2):
                            lo2, hi2 = half * 512, half * 512 + 512
                            hs2 = slice(ch * 1024 + lo2, ch * 1024 + hi2)
                            for kk in range(8):
                                nc.tensor.matmul(ps[:, lo2:hi2], wq_sb[m][:, kk, :],
                                                 xrt[:, kk, lo2:hi2],
                                                 start=(kk == 0), stop=False)
                            nc.tensor.matmul(ps[:, lo2:hi2], rs_sb[m],
                                             negmu_row[0:1, hs2],
                                             start=False, stop=True)
                        if m < 2:  # q or k: rope
                            dst = qT_sb if m == 0 else kT_sb
                            tc_t = pwork.tile([128, 1024], bf16, tag="ropec", bufs=2)
                            nc.scalar.activation(out=tc_t, in_=ps, func=AF.Copy)
                            tsw = pwork.tile([128, 1024], bf16, tag="ropesw", bufs=2)
                            for hh in range(2):
                                for a2 in range(2):
                                    nc.sync.dma_start(
                                        out=tsw[hh * 64 + a2 * 32:hh * 64 + a2 * 32 + 32, :],
                                        in_=tc_t[hh * 64 + (1 - a2) * 32:hh * 64 + (1 - a2) * 32 + 32, :])
                            nc.vector.tensor_mul(out=dst[:, chsl], in0=tc_t,
                                                 in1=tab_c[:, chsl])
                            t2 = pwork.tile([128, 1024], bf16, tag="ropet2", bufs=2)
                            nc.vector.tensor_mul(out=t2, in0=tsw, in1=tab_s[:, chsl])
                            nc.vector.tensor_add(out=dst[:, chsl], in0=dst[:, chsl],
                                                 in1=t2)
                        else:  # v: scale by rstd, transpose to [t, e] tiles
                            vt = pwork.tile([128, 1024], bf16, tag="vtmp", bufs=2)
                            nc.vector.tensor_mul(out=vt, in0=ps, in1=rsb)
                            for j in range(8):
                                g = ch * 8 + j
                                pst = psVT.tile([128, 128], bf16, tag="vtr")
                                nc.tensor.transpose(out=pst, in_=vt[:, j * 128:(j + 1) * 128],
                                                    identity=ident_bf)
                                nc.scalar.activation(out=v_all[:, g, 0:64],
                                                     in_=pst[:, 0:64], func=AF.Copy)
                                nc.scalar.activation(out=v_all[:, g, 65:129],
                                                     in_=pst[:, 64:128], func=AF.Copy)
                    if ch + 2 < NCH:
                        _load_chunk(ch + 2)

            big_pool_cm.__exit__(None, None, None)

            # =============================================================
            # Phase C: causal attention, h outer so each head's output can
            # ship through its own half AllToAll while the next head runs
            # =============================================================
            with tc.tile_pool(name="psSC", bufs=2, space="PSUM") as psSC, \
                 tc.tile_pool(name="psO", bufs=3, space="PSUM") as psO, \
                 tc.tile_pool(name="psOB", bufs=1, space="PSUM") as psOB:
                for h in range(2):
                    hsl = slice(h * 64, (h + 1) * 64)
                    for b in range(2):
                        for qc in range(4):
                            qsl = slice(b * 2048 + qc * 512, b * 2048 + (qc + 1) * 512)
                            nkt = 4 * (qc + 1)
                            ngr = nkt // 2
                            ps_o = psO.tile([65, 512], f32, tag="o")
                            pend = {}
                            for g in range(ngr + 1):
                                if g < ngr:
                                    ps_s = psSC.tile([128, 1024], f32, tag="sc")
                                    for j in range(2):
                                        kt = 2 * g + j
                                        ksl = slice(b * 2048 + kt * 128,
                                                    b * 2048 + (kt + 1) * 128)
                                        nc.tensor.matmul(
                                            ps_s[:, j * 512:(j + 1) * 512],
                                            kT_sb[hsl, ksl], qT_sb[hsl, qsl],
                                            start=True, stop=True)
                                    p_t = pwork.tile([128, 1024], bf16, tag="p", bufs=4)
                                    nc.scalar.activation(out=p_t, in_=ps_s, func=AF.Exp)
                                    for j in range(2):
                                        kt = 2 * g + j
                                        if kt >= 4 * qc:  # diagonal block: causal mask
                                            off = kt * 128 - qc * 512
                                            base = j * 512
                                            if off > 0:
                                                nc.vector.memset(p_t[:, base:base + off], 0.0)
                                            nc.vector.tensor_mul(
                                                out=p_t[:, base + off:base + off + 128],
                                                in0=p_t[:, base + off:base + off + 128],
                                                in1=mask128)
                                    pend[g] = p_t
                                # PV one group behind the scores/exp
                                ga = g - 1
                                if ga >= 0:
                                    p_t = pend.pop(ga)
                                    for j in range(2):
                                        kt = 2 * ga + j
                                        gi = b * 16 + kt
                                        nc.tensor.matmul(
                                            ps_o, v_all[:, gi, h * 65:(h + 1) * 65],
                                            p_t[:, j * 512:(j + 1) * 512],
                                            start=(kt == 0), stop=(kt == nkt - 1))
                            lns = pwork.tile([1, 512], f32, tag="lns", bufs=2)
                            nc.scalar.activation(out=lns, in_=ps_o[64:65, :],
                                                 func=AF.Ln)
                            rec16 = pwork.tile([1, 512], bf16, tag="rec16", bufs=2)
                            nc.scalar.activation(out=rec16, in_=lns,
                                                 func=AF.Exp, scale=-1.0)
                            ps_b = psOB.tile([64, 512], f32, tag="ob")
                            nc.tensor.matmul(ps_b, ones_row[0:1, 0:64], rec16,
                                             start=True, stop=True)
                            r64 = pwork.tile([64, 512], bf16, tag="r64", bufs=2)
                            nc.vector.tensor_copy(out=r64, in_=ps_b)
                            o_t = pwork.tile([64, 512], bf16, tag="o_t")
                            nc.vector.tensor_mul(out=o_t, in0=ps_o[0:64, :], in1=r64)
                            nc.sync.dma_start(out=cc_in[h][b * 4 + qc, :, :], in_=o_t)
                    # ship this head's outputs while the next head computes
                    nc.gpsimd.collective_compute(
                        "AllToAll", OP.bypass, ins=[cc_in[h][:, :, :]],
                        outs=[cc_out[h][:, :, :]],
                        replica_groups=[list(range(N_CORES))])

            attn_pool_cm.__exit__(None, None, None)
            mlp_pool_cm = tc.tile_pool(name="mlp", bufs=1)
            pmlp = mlp_pool_cm.__enter__()
            stream_pool_cm = tc.tile_pool(name="stream", bufs=2)
            pstream = stream_pool_cm.__enter__()

            # =============================================================
            # Phase D: out-proj + residual + LN2 (token-split)
            # =============================================================
            o_own = [pmlp.tile([128, 512], bf16, name=f"oo_{kk}", tag=f"oo_{kk}") for kk in range(8)]
            for kk in range(8):
                nc.sync.dma_start(out=o_own[kk][0:64, :], in_=cc_out[0][kk])
                nc.sync.dma_start(out=o_own[kk][64:128, :], in_=cc_out[1][kk])
            b1_all = pmlp.tile([128, 32], f32)
            nc.sync.dma_start(out=b1_all, in_=b1_all_d[:, :])
            b2_all = pmlp.tile([128, 8], f32)
            nc.sync.dma_start(out=b2_all, in_=b2_all_d[:, :])

            xa = [pmlp.tile([128, 512], f32, name=f"xa_{m}", tag=f"xa_{m}") for m in range(8)]
            xn = [pmlp.tile([128, 512], bf16, name=f"xn_{m}", tag=f"xn_{m}") for m in range(8)]
            with tc.tile_pool(name="psOP", bufs=2, space="PSUM") as psOP, \
                 tc.tile_pool(name="psMU", bufs=1, space="PSUM") as psMU, \
                 tc.tile_pool(name="psSQ", bufs=1, space="PSUM") as psSQ, \
                 tc.tile_pool(name="psRB", bufs=2, space="PSUM") as psRB:
                ps_mu = psMU.tile([1, 512], f32)
                ps_sq = psSQ.tile([1, 512], f32)
                xab = [None] * 8
                for m in range(8):
                    ps = psOP.tile([128, 512], f32, tag="op")
                    w = pstream.tile([128, 8, 128], bf16, tag="wo_st", bufs=3)
                    nc.sync.dma_start(out=w, in_=wout_blk[m])
                    xo = pstream.tile([128, 512], f32, tag="xo_st", bufs=3)
                    nc.sync.dma_start(out=xo, in_=xT_own[m * 128:(m + 1) * 128, :])
                    for kk in range(8):
                        nc.tensor.matmul(ps, w[:, kk, :], o_own[kk], start=(kk == 0), stop=(kk == 7))
                    nc.vector.tensor_add(out=xa[m], in0=ps, in1=xo)
                    xab[m] = pmlp.tile([128, 512], bf16, name=f"xab_{m}", tag=f"xab_{m}", bufs=1)
                    nc.vector.tensor_copy(out=xab[m], in_=xa[m])
                    sq = pwork.tile([128, 512], bf16, tag="sq", bufs=2)
                    nc.vector.tensor_mul(out=sq, in0=xab[m], in1=xab[m])
                    nc.tensor.matmul(ps_mu, ones_col, xab[m],
                                     start=(m == 0), stop=(m == 7))
                    nc.tensor.matmul(ps_sq, ones_col, sq,
                                     start=(m == 0), stop=(m == 7))

                # LN2 row stats -> center+scale the activations once
                negmu2 = pmlp.tile([1, 512], bf16)
                nc.vector.tensor_scalar_mul(out=negmu2, in0=ps_mu, scalar1=-1.0 / D)
                mus_r = pwork.tile([1, 512], f32, name="mus_r", tag="mus1_r", bufs=2)
                nc.scalar.activation(out=mus_r, in_=negmu2, func=AF.Square)
                var_r = pwork.tile([1, 512], f32, name="var_r", tag="var1_r", bufs=2)
                nc.vector.scalar_tensor_tensor(
                    out=var_r, in0=ps_sq, scalar=1.0 / D, in1=mus_r,
                    op0=OP.mult, op1=OP.subtract)
                lnv2 = pwork.tile([1, 512], f32, name="lnv2", tag="lnv", bufs=2)
                nc.scalar.activation(out=lnv2, in_=var_r, func=AF.Ln,
                                     bias=eps_col[0:1, :])
                rstd2 = pmlp.tile([1, 512], bf16)
                nc.scalar.activation(out=rstd2, in_=lnv2, func=AF.Exp, scale=-0.5)
                ps_rb = psRB.tile([128, 512], f32, tag="rb")
                nc.tensor.matmul(ps_rb, ones_row[0:1, 0:128], rstd2, start=True, stop=True)
                rstd2_sb = pmlp.tile([128, 512], bf16)
                nc.scalar.activation(out=rstd2_sb, in_=ps_rb, func=AF.Copy)
                ps_rb2 = psRB.tile([128, 512], f32, tag="rb")
                nc.tensor.matmul(ps_rb2, ones_row[0:1, 0:128], negmu2, start=True, stop=True)
                negmu2_sb = pmlp.tile([128, 512], bf16)
                nc.scalar.activation(out=negmu2_sb, in_=ps_rb2, func=AF.Copy)
                for m in range(8):
                    xc = pwork.tile([128, 512], bf16, tag="xc", bufs=2)
                    nc.vector.tensor_add(out=xc, in0=xab[m], in1=negmu2_sb)
                    nc.vector.tensor_mul(out=xn[m], in0=xc, in1=rstd2_sb)

            # =============================================================
            # Phase E: MLP (token-split, full weights, LN2 pre-applied)
            # =============================================================
            u_g = [pmlp.tile([128, 512], bf16, name=f"ug_{m}", tag=f"ug_{m}") for m in range(32)]
            with tc.tile_pool(name="psU", bufs=3, space="PSUM") as psU, \
                 tc.tile_pool(name="psDn", bufs=2, space="PSUM") as psDn:
                for m in range(32):
                    ps = psU.tile([128, 512], f32, tag="u")
                    w = pstream.tile([128, 8, 128], bf16, tag="w1_st", bufs=4)
                    nc.sync.dma_start(out=w, in_=w1_blk[m])
                    for kk in range(8):
                        nc.tensor.matmul(ps, w[:, kk, :], xn[kk], start=(kk == 0), stop=(kk == 7))
                    nc.scalar.activation(out=u_g[m], in_=ps,
                                         func=AF.Gelu_apprx_tanh, bias=b1_all[:, m:m + 1])
                for m in range(8):
                    ps = psDn.tile([128, 512], f32, tag="dn")
                    w = pstream.tile([128, 32, 128], bf16, tag="w2_st", bufs=2)
                    nc.sync.dma_start(out=w, in_=w2_blk[m])
                    for kk in range(32):
                        nc.tensor.matmul(ps, w[:, kk, :], u_g[kk], start=(kk == 0), stop=(kk == 31))
                    mt = pwork.tile([128, 512], f32, tag="mt", bufs=2)
                    nc.scalar.activation(out=mt, in_=ps, func=AF.Identity,
                                         bias=b2_all[:, m:m + 1])
                    ot = pwork.tile([128, 512], f32, tag="ot", bufs=2)
                    nc.vector.tensor_add(out=ot, in0=mt, in1=xa[m])
                    nc.sync.dma_start(out=out_d[m * 128:(m + 1) * 128, :], in_=ot)
            stream_pool_cm.__exit__(None, None, None)
            mlp_pool_cm.__exit__(None, None, None)

    _legalize_sync(nc)
    return nc


# ---------------------------------------------------------------------------
# Host-side prep + execution
# ---------------------------------------------------------------------------
_NC_CACHE = {}


def _get_nc():
    if "nc" not in _NC_CACHE:
        _NC_CACHE["nc"] = _build()
    return _NC_CACHE["nc"]


def _bf(a):
    return np.ascontiguousarray(a).astype(ml_dtypes.bfloat16)


def _f32(a):
    return np.ascontiguousarray(a, dtype=np.float32)


def _prep_inputs(x, rot_cos, rot_sin, ln1_w, w_qkv, w_out, ln2_w, w_mlp1,
                 b_mlp1, w_mlp2, b_mlp2):
    x = np.asarray(x, np.float32)
    X = x.reshape(T, D)

    xT = X.T  # (D, T)
    # (ch, p, kk, t): partition row p holds all kk-blocks contiguously
    xT_blk = _bf(xT.reshape(8, 128, NCH, 1024).transpose(2, 1, 0, 3))

    # rope tables: (128 rows = 2 heads x [first32|last32]) x T tokens
    cos = np.asarray(rot_cos, np.float32)[0, :, 0, 0, :HD // 2]  # (S, 32)
    sin = np.asarray(rot_sin, np.float32)[0, :, 0, 0, :HD // 2]
    cT = np.concatenate([cos, cos], 1).T          # (64, S)
    sT = np.concatenate([-sin, sin], 1).T         # (64, S) sign-folded
    cT = np.tile(cT, (2, B))                      # (128, T)
    sT = np.tile(sT, (2, B))
    tab = _bf(np.stack([cT, sT]))

    wqkv_eff = np.asarray(w_qkv, np.float32) * np.asarray(ln1_w, np.float32)[None, :]
    w1_eff = np.asarray(w_mlp1, np.float32) * np.asarray(ln2_w, np.float32)[None, :]
    w_out_f = np.asarray(w_out, np.float32)
    w2_f = np.asarray(w_mlp2, np.float32)

    woutT = w_out_f.T  # (d_in=head dims, e)
    wout_blk = _bf(woutT.reshape(8, 128, 8, 128).transpose(2, 1, 0, 3))  # [m, p, kk, e]
    w1T = w1_eff.T     # (D, 4D)
    w1_blk = _bf(w1T.reshape(8, 128, 32, 128).transpose(2, 1, 0, 3))
    w2T = w2_f.T       # (4D, D)
    w2_blk = _bf(w2T.reshape(32, 128, 8, 128).transpose(2, 1, 0, 3))
    b1_all = _f32(np.asarray(b_mlp1, np.float32).reshape(32, 128).T)
    b2_all = _f32(np.asarray(b_mlp2, np.float32).reshape(8, 128).T)

    in_maps = []
    for c in range(N_CORES):
        w_sl = np.concatenate(
            [wqkv_eff[0 * D + 2 * c * HD: 0 * D + 2 * (c + 1) * HD] * 0.125,
             wqkv_eff[1 * D + 2 * c * HD: 1 * D + 2 * (c + 1) * HD],
             wqkv_eff[2 * D + 2 * c * HD: 2 * D + 2 * (c + 1) * HD]], 0)  # (384, D)
        wT_sl = w_sl.T  # (D, 384) -> [m, p, kk, e]
        wqkv_b = _bf(wT_sl.reshape(8, 128, 3, 128).transpose(2, 1, 0, 3))
        wqkv_rsum = _bf(w_sl.sum(1).reshape(3, 1, 128))
        in_maps.append({
            "xT_blk": xT_blk,
            "xT_own": _f32(xT[:, c * TOK:(c + 1) * TOK]),
            "wqkv_blk": wqkv_b,
            "wqkv_rs": wqkv_rsum,
            "tab": tab,
            "wout_blk": wout_blk,
            "w1_blk": w1_blk,
            "b1_all": b1_all,
            "w2_blk": w2_blk,
            "b2_all": b2_all,
        })
    return in_maps


def _assemble(results):
    outT = np.concatenate([results[c]["out"] for c in range(N_CORES)], axis=1)
    return np.ascontiguousarray(outT.T.astype(np.float32)).reshape(B, S, D)


def run_spmd(in_maps, **kwargs):
    nc = _get_nc()
    return run_bass_kernel_spmd(nc, in_maps, core_ids=list(range(N_CORES)), **kwargs)


def kernel(x, rot_cos, rot_sin, ln1_w, w_qkv, w_out, ln2_w, w_mlp1, b_mlp1,
           w_mlp2, b_mlp2):
    in_maps = _prep_inputs(x, rot_cos, rot_sin, ln1_w, w_qkv, w_out, ln2_w,
                           w_mlp1, b_mlp1, w_mlp2, b_mlp2)
    res = run_spmd(in_maps)
    return _assemble(res.results)
